# revision 1
# baseline (speedup 1.0000x reference)
"""AGNN conv layer forward on 8 Trainium2 NeuronCores (Bass/Tile).

Strategy (matches the sharding hint): edges are sharded by destination-node
range (row_ids is sorted, so each core gets a contiguous edge range and a
12.5k-node destination range). Each core:
  1. L2-normalizes the full node table X -> Xn (bf16) into its DRAM scratch
     (replicated compute; no collectives needed).
  2. Processes its destination nodes in 128-node windows, degree-sorted so
     per-window padded degree K is tight. Neighbor features Xn[col] are
     fetched with indirect DMA gathers (one per batch of windows).
  3. Computes per-edge dots, a max-free softmax (exp(logit - |a|), exact to
     ~1e-14 relative vs the reference's segment max), and the weighted
     aggregation, all in padded [node, K] layout on the Vector/Scalar engines.
Host-side work is limited to index/layout planning (shard bounds, degree
sort, padded index arrays, masks) and un-permuting the output rows.
"""

import numpy as np

N_NODES = 100000
DIM = 32
N_CORES = 8
NPC = N_NODES // N_CORES          # destination nodes per core
NPAD = 12544                      # 98 * 128
NW = NPAD // 128                  # windows per core
XROWS = 100352                    # 8 * 12544 (X padded with zero rows)
SLOT_BUDGET = 16384               # max padded slots (128*K*B) per batch
SENT_BASE = N_NODES               # padding gathers read the zero rows

_PROGRAM_CACHE = {}


def _plan(row_ids, col_ids):
    """Host index planning. Returns per-core arrays + the shared batch schedule."""
    bounds = np.searchsorted(row_ids, np.arange(N_CORES + 1) * NPC)
    cores = []
    for c in range(N_CORES):
        s, e = int(bounds[c]), int(bounds[c + 1])
        rl = row_ids[s:e] - c * NPC
        cols = col_ids[s:e]
        deg = np.bincount(rl, minlength=NPC)
        order = np.argsort(-deg, kind="stable")          # perm: position -> local node
        deg_sorted = deg[order]
        # edge start offset per local node
        starts = np.zeros(NPC + 1, np.int64)
        np.cumsum(deg, out=starts[1:])
        cores.append(dict(s=s, cols=cols, deg=deg, order=order,
                          deg_sorted=deg_sorted, starts=starts))

    # global (shared across cores) per-window K: max over cores of window max
    Kw = np.zeros(NW, np.int64)
    for c in cores:
        ds = np.zeros(NPAD, np.int64)
        ds[:NPC] = c["deg_sorted"]
        Kw = np.maximum(Kw, ds.reshape(NW, 128).max(1))
    Kw = np.maximum(Kw, 1)

    # batches of consecutive windows, uniform K = first window's K (Kw is
    # non-increasing), bounded by SLOT_BUDGET slots
    batches = []   # (w0, B, K, col_off)
    col_off = 0
    w = 0
    while w < NW:
        K = int(Kw[w])
        B = max(1, min(NW - w, SLOT_BUDGET // (128 * K)))
        batches.append((w, B, K, col_off, tuple(int(x) for x in Kw[w:w + B])))
        col_off += B * K
        w += B
    totK = col_off

    # per-core padded arrays
    for c in cores:
        idxcat = np.full((128, totK), 0, np.int32)
        mask = np.zeros((128, totK), np.float32)
        wnidx = np.zeros((128, NW), np.int32)
        cols = c["cols"]; order = c["order"]; deg = c["deg"]; starts = c["starts"]
        pad_row = SENT_BASE + (np.arange(128, dtype=np.int32) % (XROWS - SENT_BASE))
        for (w0, B, K, off, _kws) in batches:
            for wi in range(B):
                w_ = w0 + wi
                lo = off + wi * K
                gpos = w_ * 128 + np.arange(128)          # perm positions
                valid = gpos < NPC
                nodes = np.where(valid, order[np.minimum(gpos, NPC - 1)], 0)
                wnidx[:, w_] = (nodes + cores.index(c) * 0)  # local node id
                d = np.where(valid, deg[nodes], 0).astype(np.int64)
                d = np.minimum(d, K)
                st = starts[nodes]
                # slot k of partition p: edge st[p]+k if k < d[p]
                ks = np.arange(K)
                take = ks[None, :] < d[:, None]            # [128, K]
                eidx = st[:, None] + np.minimum(ks[None, :], np.maximum(d[:, None] - 1, 0))
                vals = cols[np.minimum(eidx, len(cols) - 1)] if len(cols) else np.zeros((128, K), np.int64)
                idxcat[:, lo:lo + K] = np.where(take, vals, pad_row[:, None])
                mask[:, lo:lo + K] = take.astype(np.float32)
        # wnidx rows must be *global* X rows (destination nodes of this core)
        c["idxcat"] = idxcat
        c["mask"] = mask
        c["wnidx_local"] = wnidx
    return cores, batches, totK


def _build_program(batches, totK):
    import concourse.bass as bass
    import concourse.bacc as bacc
    import concourse.mybir as mybir
    import concourse.tile as tile

    bf = mybir.dt.bfloat16
    f32 = mybir.dt.float32
    i32 = mybir.dt.int32
    Alu = mybir.AluOpType
    Act = mybir.ActivationFunctionType

    nc = bacc.Bacc("TRN2", target_bir_lowering=False, debug=False,
                   num_devices=N_CORES, num_swdge_queues=4)
    X_in = nc.dram_tensor("X", [XROWS, DIM], f32, kind="ExternalInput").ap()
    Xw_in = nc.dram_tensor("Xw", [NPAD, DIM], f32, kind="ExternalInput").ap()
    idx_in = nc.dram_tensor("idxcat", [128, totK], i32, kind="ExternalInput").ap()
    msk_in = nc.dram_tensor("mask", [128, totK], bf, kind="ExternalInput").ap()
    aw_in = nc.dram_tensor("attw", [128, 1], f32, kind="ExternalInput").ap()
    out_d = nc.dram_tensor("out", [NPAD, DIM], f32, kind="ExternalOutput").ap()

    R = 98                       # rows per partition per normalize chunk
    CF = R * DIM                 # 3136
    x_view = X_in.rearrange("(c p r) d -> c p (r d)", c=8, p=128, r=R)
    out_view = out_d.rearrange("(w p) d -> p w d", p=128)

    def bcast(ap_, dims):
        return bass.AP(tensor=ap_.tensor, offset=ap_.offset, ap=[ap_.ap[0]] + dims)

    with tile.TileContext(nc) as tc:
        with (
            tc.tile_pool(name="norm", bufs=2) as np_pool,
            tc.tile_pool(name="small", bufs=1) as sp,
            tc.tile_pool(name="idxp", bufs=2) as idxp,
            tc.tile_pool(name="dstp", bufs=2) as dstp,
            tc.tile_pool(name="tmpp", bufs=2) as tmpp,
            tc.tile_pool(name="ep", bufs=2) as ep,
            tc.tile_pool(name="op", bufs=2) as op_,
            tc.tile_pool(name="dram", bufs=1, space="DRAM") as dramp,
        ):
            xn_t = dramp.tile([XROWS, DIM], bf)
            xn_d = xn_t[:]
            xn_view = xn_d.rearrange("(c p r) d -> c p (r d)", c=8, p=128, r=R)
            xnw_t = dramp.tile([NPAD, DIM], bf)
            xnw_wview = xnw_t[:].rearrange("(w p) d -> p w d", p=128)
            # ---- attention scalar: a and -|a| --------------------------------
            at = sp.tile([128, 1], f32)
            negC = sp.tile([128, 1], f32)
            nc.sync.dma_start(at[:], aw_in[:])
            nc.scalar.activation(out=negC[:], in_=at[:], func=Act.Abs)
            nc.vector.tensor_scalar_mul(negC[:], negC[:], -1.0)

            # ---- normalize X -> XnD (bf16), plus the window-ordered own rows -
            chunks = [(x_view[ch], xn_view[ch]) for ch in range(8)]
            chunks.append((Xw_in.rearrange("(w p) d -> p w d", p=128),
                           xnw_t[:].rearrange("(w p) d -> p w d", p=128)))
            for src_ap, dst_ap in chunks:
                xt = np_pool.tile([128, CF], f32, tag="xt")
                nc.sync.dma_start(xt[:], src_ap)
                sq = np_pool.tile([128, CF], f32, tag="sq")
                nc.scalar.activation(out=sq[:], in_=xt[:], func=Act.Square)
                ss = np_pool.tile([128, R], f32, tag="ss")
                nc.vector.tensor_reduce(
                    out=ss[:], in_=sq[:].rearrange("p (r d) -> p r d", d=DIM),
                    axis=mybir.AxisListType.X, op=Alu.add)
                nrm = np_pool.tile([128, R], f32, tag="nrm")
                nc.scalar.activation(out=nrm[:], in_=ss[:], func=Act.Sqrt)
                nc.vector.tensor_scalar_max(nrm[:], nrm[:], 1e-12)
                rn = np_pool.tile([128, R], f32, tag="rn")
                nc.vector.reciprocal(rn[:], nrm[:])
                xn = np_pool.tile([128, CF], bf, tag="xn")
                nc.vector.tensor_tensor(
                    out=xn[:].rearrange("p (r d) -> p r d", d=DIM),
                    in0=xt[:].rearrange("p (r d) -> p r d", d=DIM),
                    in1=bcast(rn[:], [[1, R], [0, DIM]]),
                    op=Alu.mult)
                nc.sync.dma_start(dst_ap, xn[:])

            # ---- main loop ---------------------------------------------------
            SMAX = max(b[1] * b[2] for b in batches)
            for bi, (w0, B, K, off, kws) in enumerate(batches):
                S = B * K
                xnw = idxp.tile([128, B * DIM], bf, tag="xnw")
                nc.sync.dma_start(
                    xnw[:].rearrange("p (b d) -> p b d", d=DIM),
                    xnw_wview[:, w0:w0 + B, :])
                idx = idxp.tile([128, S], i32, tag="idx")
                nc.sync.dma_start(idx[:], idx_in[:, off:off + S])
                msk = idxp.tile([128, S], bf, tag="msk")
                nc.sync.dma_start(msk[:], msk_in[:, off:off + S])
                dstf = dstp.tile([128, SMAX * DIM], bf, tag="dst")
                if bi < 2:
                    # first touch of each rotating buffer: clear stale bits so
                    # skipped all-pad columns never feed NaN patterns downstream
                    nc.vector.memset(dstf[:], 0.0)
                dst = dstf[:, :S * DIM]
                qn = 0
                for b in range(B):
                    for k in range(kws[b]):   # skip columns that are all-pad
                        s = b * K + k
                        gi = nc.gpsimd.indirect_dma_start(
                            out=dst[:, s * DIM:(s + 1) * DIM], out_offset=None,
                            in_=xn_d[:],
                            in_offset=bass.IndirectOffsetOnAxis(
                                ap=idx[:, s:s + 1], axis=0))
                        q = qn % 4
                        gi.ins.queue = f"qPoolDynamic{q if q else ''}"
                        qn += 1

                tmp = tmpp.tile([128, S * DIM], bf, tag="tmp")
                nc.vector.tensor_tensor(
                    out=tmp[:].rearrange("p (b k d) -> p b k d", b=B, k=K, d=DIM),
                    in0=dst.rearrange("p (b k d) -> p b k d", b=B, k=K, d=DIM),
                    in1=bcast(xnw[:], [[DIM, B], [0, K], [1, DIM]]),
                    op=Alu.mult)
                e = ep.tile([128, S], f32, tag="e")
                nc.vector.tensor_reduce(
                    out=e[:], in_=tmp[:].rearrange("p (s d) -> p s d", d=DIM),
                    axis=mybir.AxisListType.X, op=Alu.add)
                ex = ep.tile([128, S], bf, tag="ex")
                nc.scalar.activation(out=ex[:], in_=e[:], func=Act.Exp,
                                     bias=negC[:], scale=at[:])
                exm = ep.tile([128, S], bf, tag="exm")
                nc.vector.tensor_tensor(out=exm[:], in0=ex[:], in1=msk[:],
                                        op=Alu.mult)
                s_ = ep.tile([128, B], f32, tag="s")
                nc.vector.tensor_reduce(
                    out=s_[:], in_=exm[:].rearrange("p (b k) -> p b k", b=B, k=K),
                    axis=mybir.AxisListType.X, op=Alu.add)
                nc.vector.tensor_scalar_add(s_[:], s_[:], 1e-16)
                rs = ep.tile([128, B], f32, tag="rs")
                nc.vector.reciprocal(rs[:], s_[:])

                wtd = tmpp.tile([128, S * DIM], bf, tag="wtd")
                nc.vector.tensor_tensor(
                    out=wtd[:].rearrange("p (b k d) -> p b k d", b=B, k=K, d=DIM),
                    in0=dst.rearrange("p (b k d) -> p b k d", b=B, k=K, d=DIM),
                    in1=bcast(exm[:], [[K, B], [1, K], [0, DIM]]),
                    op=Alu.mult)
                acc = op_.tile([128, B * DIM], f32, tag="acc")
                nc.vector.tensor_reduce(
                    out=acc[:].rearrange("p (b d) -> p b d", b=B, d=DIM),
                    in_=bcast(wtd[:], [[K * DIM, B], [1, DIM], [DIM, K]]),
                    axis=mybir.AxisListType.X, op=Alu.add)
                o = op_.tile([128, B * DIM], f32, tag="o")
                nc.vector.tensor_tensor(
                    out=o[:].rearrange("p (b d) -> p b d", b=B, d=DIM),
                    in0=acc[:].rearrange("p (b d) -> p b d", b=B, d=DIM),
                    in1=bcast(rs[:], [[1, B], [0, DIM]]),
                    op=Alu.mult)
                nc.sync.dma_start(out_view[:, w0:w0 + B, :], o[:])
    nc.compile()
    return nc


def _warmup():
    import concourse.bacc as bacc
    import concourse.mybir as mybir
    import concourse.tile as tile
    from concourse.bass_utils import run_bass_kernel_spmd
    nc = bacc.Bacc("TRN2", target_bir_lowering=False, debug=False,
                   num_devices=N_CORES)
    x = nc.dram_tensor("x", [128, 128], mybir.dt.float32,
                       kind="ExternalInput").ap()
    y = nc.dram_tensor("y", [128, 128], mybir.dt.float32,
                       kind="ExternalOutput").ap()
    with tile.TileContext(nc) as tc:
        with tc.tile_pool(name="p", bufs=1) as pool:
            t = pool.tile([128, 128], mybir.dt.float32)
            nc.sync.dma_start(t[:], x[:])
            nc.vector.tensor_scalar_mul(t[:], t[:], 1.0)
            nc.sync.dma_start(y[:], t[:])
    nc.compile()
    a = np.zeros((128, 128), np.float32)
    run_bass_kernel_spmd(nc, [{"x": a}] * N_CORES, list(range(N_CORES)))


def kernel(X=None, weights=None, attention_w=None, row_ids=None, col_ids=None,
           n_nodes=None, **kw):
    from concourse.bass_utils import run_bass_kernel_spmd

    X = np.asarray(X, np.float32)
    attention_w = np.asarray(attention_w, np.float32)
    row_ids = np.asarray(row_ids, np.int64)
    col_ids = np.asarray(col_ids, np.int64)
    assert X.shape == (N_NODES, DIM), X.shape
    assert int(n_nodes) == N_NODES

    cores, batches, totK = _plan(row_ids, col_ids)

    key = (totK, tuple(batches))
    if key not in _PROGRAM_CACHE:
        _PROGRAM_CACHE[key] = _build_program(batches, totK)
    nc = _PROGRAM_CACHE[key]

    Xp = np.zeros((XROWS, DIM), np.float32)
    Xp[:N_NODES] = X
    aw = np.full((128, 1), attention_w.reshape(-1)[0], np.float32)

    import ml_dtypes
    in_maps = []
    for c_id, c in enumerate(cores):
        # window-ordered own destination rows: row w*128+p = X[node of
        # (partition p, window w)] (pure input row shuffle)
        wn = c["wnidx_local"].astype(np.int64) + c_id * NPC   # [128, NW]
        Xw = Xp[wn.T.reshape(-1)]                             # [NPAD, 32]
        mask_bf = c["mask"].astype(ml_dtypes.bfloat16)
        in_maps.append({
            "X": Xp, "Xw": Xw, "idxcat": c["idxcat"], "mask": mask_bf,
            "attw": aw,
        })
    try:
        res = run_bass_kernel_spmd(nc, in_maps, list(range(N_CORES)))
    except Exception:
        # A previously wedged exec unit sometimes needs one trivial run to
        # recover; retry once after a warm-up kernel.
        try:
            _warmup()
        except Exception:
            pass
        res = run_bass_kernel_spmd(nc, in_maps, list(range(N_CORES)))

    out = np.zeros((N_NODES, DIM), np.float32)
    for c_id, c in enumerate(cores):
        oc = res.results[c_id]["out"]          # [NPAD, 32] in perm order
        order = c["order"]
        out[c_id * NPC + order] = oc[:NPC]
    return out

